# revision 1
# baseline (speedup 1.0000x reference)
"""EnsembleRSSM observe kernel for 8 Trainium2 NeuronCores.

Strategy: data-parallel over batch (B=128 -> 16 rows/core), zero cross-core
communication. The sequential T=64 scan runs per-core on its batch shard with
all scan weights resident in SBUF (bf16 matmul operands, fp32 PSUM/vector
math). The embed-dependent part of the posterior MLP (embed @ We) is
precomputed in a batched phase 0 at full PE utilization (rows = T*16). The
ensemble prior head is decoupled from the recurrence and computed in a
batched phase 2, with timesteps host-sorted by ensemble index so each head's
weights are loaded once and matmul row-tiles are dense.
"""
import sys

sys.path.insert(0, "/opt/trn_rl_repo")

import numpy as np
import ml_dtypes

import concourse.bass as bass
import concourse.bacc as bacc
import concourse.mybir as mybir
import concourse.tile as tile
from concourse import bass_utils

_orig_get_tables = bacc.get_activation_tables

def _filtered_tables(arch):
    tabs = _orig_get_tables(arch)
    keep = ("natural_log_exp_and_others", "sigmoid_and_others")
    if not all(k in tabs for k in keep):
        return tabs
    return {k: (v if k in keep else set()) for k, v in tabs.items()}

bacc.get_activation_tables = _filtered_tables

B, T = 128, 64
EMB, ACT = 1536, 12
STOCH, DETER, HIDDEN, ENS = 32, 1024, 1024, 5
MIN_STD = 0.1
UPDATE_BIAS = -1.0
NC = 8
BL = B // NC  # 16 rows per core

F32 = mybir.dt.float32
BF16 = mybir.dt.bfloat16
AF = mybir.ActivationFunctionType
ALU = mybir.AluOpType
AX = mybir.AxisListType.X

_CACHE = {}
_LAST_IN_MAPS = None


def _row_layout(ens_index):
    """Head-sorted, 128-padded row layout for phase 2."""
    order = {}
    for t in range(T):
        order.setdefault(int(ens_index[t]), []).append(t)
    row_ofs = [0] * T
    heads = []
    r = 0
    for k in range(ENS):
        ts = order.get(k, [])
        r0 = r
        for t in ts:
            row_ofs[t] = r
            r += BL
        r = ((r + 127) // 128) * 128
        heads.append((k, r0, r, ts))
    return row_ofs, heads, max(r, 128)


def _build(ens_index, affine_trivial):
    nc = bacc.Bacc("TRN2", target_bir_lowering=False, debug=False,
                   num_devices=NC, detect_race_conditions=False)
    for cval in (-1.0, -2.0, -0.5, 1e-5):
        th = nc.alloc_sbuf_tensor(f"constx-{cval}", [128, 1], F32)
        nc.gpsimd.memset(th.ap(), cval)
        nc.const_aps.aps[(F32, cval)] = th.ap()

    # ---- DRAM inputs ----
    d_embT = nc.dram_tensor("embT", [EMB, T * BL], BF16, kind="ExternalInput")
    d_We = nc.dram_tensor("We", [EMB, HIDDEN], BF16, kind="ExternalInput")
    d_Wg = nc.dram_tensor("Wg", [HIDDEN + DETER, 3 * DETER], BF16, kind="ExternalInput")
    d_Wd = nc.dram_tensor("Wd", [DETER, HIDDEN], BF16, kind="ExternalInput")
    d_Wimg = nc.dram_tensor("Wimg", [STOCH + ACT + 1, HIDDEN], BF16, kind="ExternalInput")
    d_Wod = nc.dram_tensor("Wod", [HIDDEN, 2 * STOCH], BF16, kind="ExternalInput")
    d_Wio = nc.dram_tensor("Wio", [ENS, DETER, HIDDEN], BF16, kind="ExternalInput")
    d_Wids = nc.dram_tensor("Wids", [ENS, HIDDEN, 2 * STOCH], BF16, kind="ExternalInput")
    d_masks = nc.dram_tensor("masks", [BL, T + 1], F32, kind="ExternalInput")
    d_am1 = nc.dram_tensor("am1", [BL, T * (ACT + 1)], F32, kind="ExternalInput")
    d_eye = nc.dram_tensor("eye16", [BL, BL], F32, kind="ExternalInput")
    d_eye128 = nc.dram_tensor("eye128", [128, 128], F32, kind="ExternalInput")
    d_gbn_g = nc.dram_tensor("gbn_gru", [BL, 2 * 3 * DETER], F32, kind="ExternalInput")
    d_gbn_o = nc.dram_tensor("gbn_obs", [BL, 2 * HIDDEN], F32, kind="ExternalInput")
    d_gbn_i = nc.dram_tensor("gbn_img", [BL, 2 * HIDDEN], F32, kind="ExternalInput")
    d_gbn_e = nc.dram_tensor("gbn_ens", [ENS, 128, 2 * HIDDEN], F32, kind="ExternalInput")
    d_bias_o = nc.dram_tensor("bias_obs", [128, HIDDEN], F32, kind="ExternalInput")
    d_bias_e = nc.dram_tensor("bias_ens", [ENS, 128, HIDDEN], F32, kind="ExternalInput")
    d_bod = nc.dram_tensor("b_od", [BL, 2 * STOCH], F32, kind="ExternalInput")
    d_bids = nc.dram_tensor("b_ids", [ENS, 128, 2 * STOCH], F32, kind="ExternalInput")

    o_deter = nc.dram_tensor("o_deter", [BL, T, DETER], F32, kind="ExternalOutput")
    o_dist = nc.dram_tensor("o_dist", [BL, T, 2 * STOCH], F32, kind="ExternalOutput")
    o_ens = nc.dram_tensor("o_ens", [BL, T, 2 * STOCH], F32, kind="ExternalOutput")

    row_ofs, heads, R = _row_layout(ens_index)
    s_xop = nc.dram_tensor("xo_pre", [T * BL, HIDDEN], F32)
    s_dTa = nc.dram_tensor("deterT_all", [8, 128, R], BF16)

    KT_G = (HIDDEN + DETER) // 128  # 16
    NT_G = (3 * DETER) // 512       # 6
    KT_D = DETER // 128             # 8
    KT_E = EMB // 128               # 12

    with tile.TileContext(nc) as tc:
        with tc.tile_pool(name="wpool", bufs=1) as wpool, \
             tc.tile_pool(name="spool", bufs=1) as spool, \
             tc.tile_pool(name="spool2", bufs=2) as spool2, \
             tc.tile_pool(name="ppool", bufs=6, space="PSUM") as ppool, \
             tc.tile_pool(name="tpool", bufs=2, space="PSUM") as tpool:

            # ---------- resident weights ----------
            Wg = wpool.tile([128, KT_G * 3 * DETER], BF16, tag="Wg")
            for k in range(KT_G):
                nc.sync.dma_start(Wg[:, k * 3072:(k + 1) * 3072],
                                  d_Wg.ap()[k * 128:(k + 1) * 128, :])
            Wd = wpool.tile([128, KT_D * HIDDEN], BF16, tag="Wd")
            for k in range(KT_D):
                nc.sync.dma_start(Wd[:, k * 1024:(k + 1) * 1024],
                                  d_Wd.ap()[k * 128:(k + 1) * 128, :])
            Wimg = wpool.tile([STOCH + ACT + 1, HIDDEN], BF16, tag="Wimg")
            nc.sync.dma_start(Wimg[:], d_Wimg.ap())
            Wod = wpool.tile([128, KT_D * 2 * STOCH], BF16, tag="Wod")
            for k in range(KT_D):
                nc.sync.dma_start(Wod[:, k * 64:(k + 1) * 64],
                                  d_Wod.ap()[k * 128:(k + 1) * 128, :])
            eye = wpool.tile([BL, BL], F32, tag="eye")
            nc.sync.dma_start(eye[:], d_eye.ap())
            eye128 = wpool.tile([128, 128], F32, tag="eye128")
            nc.sync.dma_start(eye128[:], d_eye128.ap())
            masks = wpool.tile([BL, T + 1], F32, tag="masks")
            nc.sync.dma_start(masks[:], d_masks.ap())
            am1 = wpool.tile([BL, T * (ACT + 1)], F32, tag="am1")
            nc.sync.dma_start(am1[:], d_am1.ap())
            if not affine_trivial:
                gbn_g = wpool.tile([BL, 2 * 3 * DETER], F32, tag="gg")
                nc.sync.dma_start(gbn_g[:], d_gbn_g.ap())
                gbn_o = wpool.tile([BL, 2 * HIDDEN], F32, tag="go")
                nc.sync.dma_start(gbn_o[:], d_gbn_o.ap())
                gbn_i = wpool.tile([BL, 2 * HIDDEN], F32, tag="gi")
                nc.sync.dma_start(gbn_i[:], d_gbn_i.ap())
                bod = wpool.tile([BL, 2 * STOCH], F32, tag="bod")
                nc.sync.dma_start(bod[:], d_bod.ap())

            # ---------- phase 0 ----------
            with tc.tile_pool(name="p0w", bufs=1) as p0w, \
                 tc.tile_pool(name="p0e", bufs=13) as p0e, \
                 tc.tile_pool(name="p0v", bufs=3) as p0v:
                bias_o_sb = None
                if not affine_trivial:
                    bias_o_sb = p0w.tile([128, HIDDEN], F32, tag="biaso")
                    nc.sync.dma_start(bias_o_sb[:], d_bias_o.ap())
                for mt in range(T * BL // 128):
                    ek = []
                    for k in range(KT_E):
                        ekt = p0e.tile([128, 128], BF16, tag="p0ek")
                        nc.sync.dma_start(ekt[:], d_embT.ap()[k * 128:(k + 1) * 128,
                                                              mt * 128:(mt + 1) * 128])
                        ek.append(ekt)
                    xop0 = spool.tile([128, HIDDEN], F32, tag="p0out")
                    wek = []
                    for k in range(KT_E):
                        wt = p0v.tile([128, HIDDEN], BF16, tag="p0we")
                        nc.sync.dma_start(wt[:], d_We.ap()[k * 128:(k + 1) * 128, :])
                        wek.append(wt)
                    for nt in range(2):
                        ps = ppool.tile([128, 512], F32, tag="ps")
                        for k in range(KT_E):
                            nc.tensor.matmul(
                                ps[:], ek[k][:],
                                wek[k][:, nt * 512:(nt + 1) * 512],
                                start=(k == 0), stop=(k == KT_E - 1))
                        if bias_o_sb is not None:
                            nc.vector.tensor_add(xop0[:, nt * 512:(nt + 1) * 512], ps[:],
                                                 bias_o_sb[:, nt * 512:(nt + 1) * 512])
                        else:
                            nc.scalar.copy(xop0[:, nt * 512:(nt + 1) * 512], ps[:])
                    nc.sync.dma_start(s_xop.ap()[mt * 128:(mt + 1) * 128, :], xop0[:])

                # zero the pad regions of deterT_all
                zpad = spool.tile([128, 128], BF16, tag="zpad")
                nc.vector.memset(zpad[:], 0.0)
                for (_k, r0, r1, ts) in heads:
                    pr0 = r0 + len(ts) * BL
                    if pr0 < r1:
                        for kk in range(8):
                            nc.sync.dma_start(s_dTa.ap()[kk, :, pr0:r1], zpad[:, 0:r1 - pr0])

            # ---------- scan ----------

            def ln_stats(stats, ncols, scol, qcol, width, tag):
                s = spool2.tile([BL, 4], F32, tag=tag)
                nc.vector.reduce_sum(s[:, 0:1], stats[:, scol:scol + ncols], axis=AX)
                nc.vector.reduce_sum(s[:, 1:2], stats[:, qcol:qcol + ncols], axis=AX)
                nc.vector.tensor_scalar_mul(s[:, 0:1], s[:, 0:1], 1.0 / width)
                nc.vector.tensor_scalar_mul(s[:, 1:2], s[:, 1:2], 1.0 / width)
                nc.vector.tensor_mul(s[:, 2:3], s[:, 0:1], s[:, 0:1])
                nc.vector.tensor_sub(s[:, 1:2], s[:, 1:2], s[:, 2:3])
                nc.scalar.activation(s[:, 1:2], s[:, 1:2], AF.Ln, bias=1e-5)
                nc.scalar.activation(s[:, 3:4], s[:, 1:2], AF.Exp, scale=-0.5)
                nc.vector.tensor_mul(s[:, 2:3], s[:, 0:1], s[:, 3:4])
                return s

            def elu(dst, src, width, parts):
                t1 = spool.tile([parts, width], F32, tag="elu_t")
                nc.vector.tensor_scalar_min(t1[:], src, 0.0)
                nc.scalar.activation(t1[:], t1[:], AF.Exp)
                nc.scalar.activation(dst, src, AF.Relu)
                nc.vector.tensor_add(dst, dst, t1[:])
                nc.vector.tensor_scalar_add(dst, dst, -1.0)

            deter = spool2.tile([BL, DETER], F32, tag="deterN")
            nc.vector.memset(deter[:], 0.0)
            deterT = spool2.tile([128, 128], BF16, tag="deterT")
            nc.vector.memset(deterT[:], 0.0)
            in45 = spool2.tile([BL, STOCH + ACT + 1], F32, tag="in45")
            nc.vector.memset(in45[:, 0:STOCH], 0.0)
            nc.vector.tensor_copy(in45[:, STOCH:], am1[:, 0:ACT + 1])

            for t in range(T):
                m_t = masks[:, t:t + 1]
                # --- img_in MLP ---
                tp = tpool.tile([128, 128], F32, tag="tp")
                nc.tensor.transpose(tp[0:STOCH + ACT + 1, 0:BL], in45[:], eye[:])
                in45T = spool.tile([STOCH + ACT + 1, BL], BF16, tag="in45T")
                nc.vector.tensor_copy(in45T[:], tp[0:STOCH + ACT + 1, 0:BL])
                z_i = spool.tile([BL, HIDDEN], F32, tag="z_i")
                stats_i = spool2.tile([BL, 4], F32, tag="stats_i")
                for nt in range(2):
                    ps = ppool.tile([BL, 512], F32, tag="ps")
                    nc.tensor.matmul(ps[:], in45T[:], Wimg[:, nt * 512:(nt + 1) * 512],
                                     start=True, stop=True)
                    nc.scalar.activation(z_i[:, nt * 512:(nt + 1) * 512], ps[:], AF.Copy,
                                         accum_out=stats_i[:, nt:nt + 1])
                    sqt = spool.tile([BL, 512], F32, tag="sqt")
                    nc.scalar.activation(sqt[:], ps[:], AF.Square,
                                         accum_out=stats_i[:, 2 + nt:3 + nt])
                si = ln_stats(stats_i, 2, 0, 2, HIDDEN, "st_i")
                xln = spool.tile([BL, HIDDEN], F32, tag="xln")
                nc.vector.tensor_scalar(xln[:], z_i[:], si[:, 3:4], si[:, 2:3],
                                        ALU.mult, ALU.subtract)
                if not affine_trivial:
                    nc.vector.tensor_mul(xln[:], xln[:], gbn_i[:, 0:HIDDEN])
                    nc.vector.tensor_add(xln[:], xln[:], gbn_i[:, HIDDEN:])
                x = xln
                elu(x[:], xln[:], HIDDEN, BL)
                xT = spool.tile([128, 128], BF16, tag="xT")
                for s8 in range(8):
                    tpx = tpool.tile([128, 128], F32, tag="tp")
                    nc.tensor.transpose(tpx[:, 0:BL], x[:, s8 * 128:(s8 + 1) * 128], eye[:])
                    nc.vector.tensor_copy(xT[:, s8 * BL:(s8 + 1) * BL], tpx[:, 0:BL])

                # --- GRU ---
                z_g = spool.tile([BL, 3 * DETER], F32, tag="z_g")
                stats_g = spool2.tile([BL, 16], F32, tag="stats_g")
                for nt in range(NT_G):
                    psx = ppool.tile([BL, 512], F32, tag="ps")
                    psd = ppool.tile([BL, 512], F32, tag="ps")
                    for k in range(8):
                        nc.tensor.matmul(psx[:], xT[:, k * BL:(k + 1) * BL],
                                         Wg[:, k * 3072 + nt * 512: k * 3072 + nt * 512 + 512],
                                         start=(k == 0), stop=(k == 7))
                    for k in range(8):
                        nc.tensor.matmul(psd[:], deterT[:, k * BL:(k + 1) * BL],
                                         Wg[:, (8 + k) * 3072 + nt * 512: (8 + k) * 3072 + nt * 512 + 512],
                                         start=(k == 0), stop=(k == 7))
                    nc.scalar.copy(z_g[:, nt * 512:(nt + 1) * 512], psx[:])
                    nc.vector.scalar_tensor_tensor(
                        z_g[:, nt * 512:(nt + 1) * 512], psd[:], m_t,
                        z_g[:, nt * 512:(nt + 1) * 512],
                        ALU.mult, ALU.add, accum_out=stats_g[:, nt:nt + 1])
                    sqg = spool.tile([BL, 512], F32, tag="sqt")
                    nc.scalar.activation(sqg[:], z_g[:, nt * 512:(nt + 1) * 512], AF.Square,
                                         accum_out=stats_g[:, 8 + nt:9 + nt])
                sg = ln_stats(stats_g, 6, 0, 8, 3 * DETER, "st_g")
                zn = z_g
                nc.vector.tensor_scalar(zn[:], z_g[:], sg[:, 3:4], sg[:, 2:3],
                                        ALU.mult, ALU.subtract)
                if not affine_trivial:
                    nc.vector.tensor_mul(zn[:], zn[:], gbn_g[:, 0:3 * DETER])
                    nc.vector.tensor_add(zn[:], zn[:], gbn_g[:, 3 * DETER:])
                gate = spool.tile([BL, DETER], F32, tag="gate")
                nc.scalar.activation(gate[:], zn[:, 0:DETER], AF.Sigmoid)
                nc.vector.tensor_mul(gate[:], gate[:], zn[:, DETER:2 * DETER])
                nc.scalar.activation(gate[:], gate[:], AF.Tanh)
                upd = spool.tile([BL, DETER], F32, tag="upd")
                nc.scalar.activation(upd[:], zn[:, 2 * DETER:], AF.Sigmoid, bias=UPDATE_BIAS)
                dm = spool.tile([BL, DETER], F32, tag="dm")
                nc.vector.tensor_scalar_mul(dm[:], deter[:], m_t)
                deter = spool2.tile([BL, DETER], F32, tag="deterN")
                nc.vector.tensor_sub(gate[:], gate[:], dm[:])
                nc.vector.tensor_mul(gate[:], gate[:], upd[:])
                nc.vector.tensor_add(deter[:], dm[:], gate[:])
                nc.sync.dma_start(o_deter.ap()[:, t, :], deter[:])
                deterT = spool2.tile([128, 128], BF16, tag="deterT")
                for s8 in range(8):
                    tpd = tpool.tile([128, 128], F32, tag="tp")
                    nc.tensor.transpose(tpd[:, 0:BL], deter[:, s8 * 128:(s8 + 1) * 128], eye[:])
                    nc.vector.tensor_copy(deterT[:, s8 * BL:(s8 + 1) * BL], tpd[:, 0:BL])
                    nc.sync.dma_start(s_dTa.ap()[s8, :, row_ofs[t]:row_ofs[t] + BL],
                                      deterT[:, s8 * BL:(s8 + 1) * BL])

                # --- obs posterior ---
                xop = spool.tile([BL, HIDDEN], F32, tag="xopS")
                nc.sync.dma_start(xop[:], s_xop.ap()[t * BL:(t + 1) * BL, :])
                z_o = spool.tile([BL, HIDDEN], F32, tag="z_o")
                stats_o = spool2.tile([BL, 4], F32, tag="stats_o")
                for nt in range(2):
                    ps = ppool.tile([BL, 512], F32, tag="ps")
                    for k in range(8):
                        nc.tensor.matmul(ps[:], deterT[:, k * BL:(k + 1) * BL],
                                         Wd[:, k * 1024 + nt * 512: k * 1024 + nt * 512 + 512],
                                         start=(k == 0), stop=(k == 7))
                    nc.vector.scalar_tensor_tensor(
                        z_o[:, nt * 512:(nt + 1) * 512], ps[:], 1.0,
                        xop[:, nt * 512:(nt + 1) * 512],
                        ALU.mult, ALU.add, accum_out=stats_o[:, nt:nt + 1])
                    sqo = spool.tile([BL, 512], F32, tag="sqt")
                    nc.scalar.activation(sqo[:], z_o[:, nt * 512:(nt + 1) * 512], AF.Square,
                                         accum_out=stats_o[:, 2 + nt:3 + nt])
                so = ln_stats(stats_o, 2, 0, 2, HIDDEN, "st_o")
                oln = z_o
                nc.vector.tensor_scalar(oln[:], z_o[:], so[:, 3:4], so[:, 2:3],
                                        ALU.mult, ALU.subtract)
                if not affine_trivial:
                    nc.vector.tensor_mul(oln[:], oln[:], gbn_o[:, 0:HIDDEN])
                    nc.vector.tensor_add(oln[:], oln[:], gbn_o[:, HIDDEN:])
                xo = oln
                elu(xo[:], oln[:], HIDDEN, BL)
                xoT = spool.tile([128, 128], BF16, tag="xoT")
                for s8 in range(8):
                    tpo = tpool.tile([128, 128], F32, tag="tp")
                    nc.tensor.transpose(tpo[:, 0:BL], xo[:, s8 * 128:(s8 + 1) * 128], eye[:])
                    nc.vector.tensor_copy(xoT[:, s8 * BL:(s8 + 1) * BL], tpo[:, 0:BL])
                psd2 = ppool.tile([BL, 2 * STOCH], F32, tag="ps")
                for k in range(8):
                    nc.tensor.matmul(psd2[:], xoT[:, k * BL:(k + 1) * BL],
                                     Wod[:, k * 64:(k + 1) * 64],
                                     start=(k == 0), stop=(k == 7))
                dist = spool.tile([BL, 2 * STOCH], F32, tag="dist")
                nc.vector.tensor_copy(dist[:], psd2[:])
                if not affine_trivial:
                    nc.vector.tensor_add(dist[:], dist[:], bod[:])
                outd = spool.tile([BL, 2 * STOCH], F32, tag="outd")
                nc.vector.tensor_copy(outd[:, 0:STOCH], dist[:, 0:STOCH])
                nc.scalar.activation(outd[:, STOCH:], dist[:, STOCH:], AF.Exp)
                nc.vector.tensor_scalar_add(outd[:, STOCH:], outd[:, STOCH:], 1.0)
                nc.scalar.activation(outd[:, STOCH:], outd[:, STOCH:], AF.Ln)
                nc.vector.tensor_scalar_add(outd[:, STOCH:], outd[:, STOCH:], MIN_STD)
                nc.sync.dma_start(o_dist.ap()[:, t, :], outd[:])
                in45 = spool2.tile([BL, STOCH + ACT + 1], F32, tag="in45")
                if t + 1 < T:
                    nc.vector.tensor_scalar_mul(in45[:, 0:STOCH], dist[:, 0:STOCH],
                                                masks[:, t + 1:t + 2])
                    nc.vector.tensor_copy(in45[:, STOCH:],
                                          am1[:, (t + 1) * (ACT + 1):(t + 2) * (ACT + 1)])

            # ---------- phase 2: ensemble priors ----------
            with tc.tile_pool(name="p2w", bufs=1) as p2w, \
                 tc.tile_pool(name="p2d", bufs=10) as p2d:
                for (k, r0, r1, ts) in heads:
                    if not ts:
                        continue
                    Wio = p2w.tile([128, KT_D * HIDDEN], BF16, tag="Wio")
                    for kk in range(KT_D):
                        nc.sync.dma_start(Wio[:, kk * 1024:(kk + 1) * 1024],
                                          d_Wio.ap()[k, kk * 128:(kk + 1) * 128, :])
                    Wids = p2w.tile([128, KT_D * 2 * STOCH], BF16, tag="Wids")
                    for kk in range(KT_D):
                        nc.sync.dma_start(Wids[:, kk * 64:(kk + 1) * 64],
                                          d_Wids.ap()[k, kk * 128:(kk + 1) * 128, :])
                    if not affine_trivial:
                        gbn_e = p2w.tile([128, 2 * HIDDEN], F32, tag="gbnE")
                        nc.sync.dma_start(gbn_e[:], d_gbn_e.ap()[k])
                        bias_e = p2w.tile([128, HIDDEN], F32, tag="biasE")
                        nc.sync.dma_start(bias_e[:], d_bias_e.ap()[k])
                        bids = p2w.tile([128, 2 * STOCH], F32, tag="bidsE")
                        nc.sync.dma_start(bids[:], d_bids.ap()[k])
                    for mt in range(r0 // 128, r1 // 128):
                        dk = []
                        for kk in range(8):
                            dkt = p2d.tile([128, 128], BF16, tag="p2dk")
                            nc.sync.dma_start(dkt[:], s_dTa.ap()[kk, :, mt * 128:(mt + 1) * 128])
                            dk.append(dkt)
                        z_e = spool.tile([128, HIDDEN], F32, tag="z_g")
                        estats = spool.tile([128, 4], F32, tag="estats")
                        for nt in range(2):
                            ps = ppool.tile([128, 512], F32, tag="ps")
                            for kk in range(8):
                                nc.tensor.matmul(
                                    ps[:], dk[kk][:],
                                    Wio[:, kk * 1024 + nt * 512: kk * 1024 + nt * 512 + 512],
                                    start=(kk == 0), stop=(kk == 7))
                            if not affine_trivial:
                                nc.vector.tensor_add(z_e[:, nt * 512:(nt + 1) * 512], ps[:],
                                                     bias_e[:, nt * 512:(nt + 1) * 512])
                            else:
                                nc.scalar.copy(z_e[:, nt * 512:(nt + 1) * 512], ps[:])
                            nc.vector.reduce_sum(estats[:, nt:nt + 1],
                                                 z_e[:, nt * 512:(nt + 1) * 512], axis=AX)
                            sqe = spool.tile([128, 512], F32, tag="sqt")
                            nc.scalar.activation(sqe[:], z_e[:, nt * 512:(nt + 1) * 512],
                                                 AF.Square, accum_out=estats[:, 2 + nt:3 + nt])
                        se = spool.tile([128, 4], F32, tag="lnst")
                        nc.vector.reduce_sum(se[:, 0:1], estats[:, 0:2], axis=AX)
                        nc.vector.reduce_sum(se[:, 1:2], estats[:, 2:4], axis=AX)
                        nc.vector.tensor_scalar_mul(se[:, 0:1], se[:, 0:1], 1.0 / HIDDEN)
                        nc.vector.tensor_scalar_mul(se[:, 1:2], se[:, 1:2], 1.0 / HIDDEN)
                        nc.vector.tensor_mul(se[:, 2:3], se[:, 0:1], se[:, 0:1])
                        nc.vector.tensor_sub(se[:, 1:2], se[:, 1:2], se[:, 2:3])
                        nc.scalar.activation(se[:, 1:2], se[:, 1:2], AF.Ln, bias=1e-5)
                        nc.scalar.activation(se[:, 3:4], se[:, 1:2], AF.Exp, scale=-0.5)
                        nc.vector.tensor_mul(se[:, 2:3], se[:, 0:1], se[:, 3:4])
                        eln = z_e
                        nc.vector.tensor_scalar(eln[:], z_e[:], se[:, 3:4], se[:, 2:3],
                                                ALU.mult, ALU.subtract)
                        if not affine_trivial:
                            nc.vector.tensor_mul(eln[:], eln[:], gbn_e[:, 0:HIDDEN])
                            nc.vector.tensor_add(eln[:], eln[:], gbn_e[:, HIDDEN:])
                        hs = eln
                        elu(hs[:], eln[:], HIDDEN, 128)
                        hsT = spool.tile([128, 8 * 128], BF16, tag="hsT")
                        for s8 in range(8):
                            tpe = tpool.tile([128, 128], F32, tag="tp")
                            nc.tensor.transpose(tpe[:], hs[:, s8 * 128:(s8 + 1) * 128], eye128[:])
                            nc.vector.tensor_copy(hsT[:, s8 * 128:(s8 + 1) * 128], tpe[:])
                        pse = ppool.tile([128, 2 * STOCH], F32, tag="ps")
                        for kk in range(8):
                            nc.tensor.matmul(pse[:], hsT[:, kk * 128:(kk + 1) * 128],
                                             Wids[:, kk * 64:(kk + 1) * 64],
                                             start=(kk == 0), stop=(kk == 7))
                        ed = spool.tile([128, 2 * STOCH], F32, tag="ed")
                        nc.vector.tensor_copy(ed[:], pse[:])
                        if not affine_trivial:
                            nc.vector.tensor_add(ed[:], ed[:], bids[:])
                        eo = spool.tile([128, 2 * STOCH], F32, tag="eo")
                        nc.vector.tensor_copy(eo[:, 0:STOCH], ed[:, 0:STOCH])
                        nc.scalar.activation(eo[:, STOCH:], ed[:, STOCH:], AF.Exp)
                        nc.vector.tensor_scalar_add(eo[:, STOCH:], eo[:, STOCH:], 1.0)
                        nc.scalar.activation(eo[:, STOCH:], eo[:, STOCH:], AF.Ln)
                        nc.vector.tensor_scalar_add(eo[:, STOCH:], eo[:, STOCH:], MIN_STD)
                        for t in ts:
                            if row_ofs[t] // 128 == mt:
                                lr = row_ofs[t] - mt * 128
                                nc.sync.dma_start(o_ens.ap()[:, t, :], eo[lr:lr + BL, :])

    nc.compile()
    return nc


def kernel(embed, action, is_first, ens_index,
           W_img_in, b_img_in, g_img_in, bn_img_in,
           W_gru, b_gru, g_gru, bn_gru,
           W_img_out, b_img_out, g_img_out, bn_img_out,
           W_img_dist, b_img_dist,
           W_obs_out, b_obs_out, g_obs_out, bn_obs_out,
           W_obs_dist, b_obs_dist):
    embed = np.asarray(embed, np.float32)
    action = np.asarray(action, np.float32)
    is_first = np.asarray(is_first)
    ens_index = np.asarray(ens_index, np.int64)

    affine_trivial = bool(
        np.all(np.asarray(b_img_in) == 0) and np.all(np.asarray(g_img_in) == 1)
        and np.all(np.asarray(bn_img_in) == 0)
        and np.all(np.asarray(b_gru) == 0) and np.all(np.asarray(g_gru) == 1)
        and np.all(np.asarray(bn_gru) == 0)
        and np.all(np.asarray(b_img_out) == 0) and np.all(np.asarray(g_img_out) == 1)
        and np.all(np.asarray(bn_img_out) == 0)
        and np.all(np.asarray(b_img_dist) == 0)
        and np.all(np.asarray(b_obs_out) == 0) and np.all(np.asarray(g_obs_out) == 1)
        and np.all(np.asarray(bn_obs_out) == 0)
        and np.all(np.asarray(b_obs_dist) == 0))

    key = (tuple(int(x) for x in ens_index), affine_trivial)
    if key not in _CACHE:
        _CACHE[key] = _build(ens_index, affine_trivial)
    nc = _CACHE[key]

    bf = lambda a: np.ascontiguousarray(np.asarray(a, np.float32)).astype(ml_dtypes.bfloat16)
    mask = 1.0 - np.asarray(is_first, np.float32)          # [B, T]
    a_m = action * mask[:, :, None]                        # [B, T, ACT]
    Wimg = np.concatenate([np.asarray(W_img_in, np.float32),
                           np.asarray(b_img_in, np.float32)[None, :]], 0)

    rep = {
        "We": bf(W_obs_out[DETER:]), "Wd": bf(W_obs_out[:DETER]),
        "Wg": bf(W_gru), "Wimg": bf(Wimg), "Wod": bf(W_obs_dist),
        "Wio": bf(W_img_out), "Wids": bf(W_img_dist),
        "eye16": np.eye(BL, dtype=np.float32),
        "eye128": np.eye(128, dtype=np.float32),
        "gbn_gru": np.tile(np.concatenate([np.asarray(g_gru, np.float32),
                                           np.asarray(bn_gru, np.float32)])[None, :], (BL, 1)),
        "gbn_obs": np.tile(np.concatenate([np.asarray(g_obs_out, np.float32),
                                           np.asarray(bn_obs_out, np.float32)])[None, :], (BL, 1)),
        "gbn_img": np.tile(np.concatenate([np.asarray(g_img_in, np.float32),
                                           np.asarray(bn_img_in, np.float32)])[None, :], (BL, 1)),
        "gbn_ens": np.ascontiguousarray(np.tile(np.concatenate(
            [np.asarray(g_img_out, np.float32), np.asarray(bn_img_out, np.float32)], 1)[:, None, :],
            (1, 128, 1))),
        "bias_obs": np.tile(np.asarray(b_obs_out, np.float32)[None, :], (128, 1)),
        "bias_ens": np.ascontiguousarray(
            np.tile(np.asarray(b_img_out, np.float32)[:, None, :], (1, 128, 1))),
        "b_od": np.tile(np.asarray(b_obs_dist, np.float32)[None, :], (BL, 1)),
        "b_ids": np.ascontiguousarray(
            np.tile(np.asarray(b_img_dist, np.float32)[:, None, :], (1, 128, 1))),
    }
    in_maps = []
    for j in range(NC):
        sl = slice(j * BL, (j + 1) * BL)
        e = embed[sl]                                      # [16, T, EMB]
        embT = np.ascontiguousarray(e.transpose(2, 1, 0).reshape(EMB, T * BL))
        am1 = np.concatenate([a_m[sl], np.ones((BL, T, 1), np.float32)], -1)
        masks = np.concatenate([mask[sl], np.ones((BL, 1), np.float32)], 1)
        im = dict(rep)
        im["embT"] = bf(embT)
        im["am1"] = np.ascontiguousarray(am1.reshape(BL, T * (ACT + 1)).astype(np.float32))
        im["masks"] = np.ascontiguousarray(masks.astype(np.float32))
        in_maps.append(im)

    global _LAST_IN_MAPS
    _LAST_IN_MAPS = in_maps
    res = bass_utils.run_bass_kernel_spmd(nc, in_maps, core_ids=list(range(NC)))

    o_deter = np.concatenate([res.results[j]["o_deter"] for j in range(NC)], 0)
    o_dist = np.concatenate([res.results[j]["o_dist"] for j in range(NC)], 0)
    o_ens = np.concatenate([res.results[j]["o_ens"] for j in range(NC)], 0)

    omean, ostd = o_dist[..., :STOCH], o_dist[..., STOCH:]
    pmean, pstd = o_ens[..., :STOCH], o_ens[..., STOCH:]
    out = np.concatenate([omean, ostd, omean, pmean, pstd, pmean, o_deter], -1)
    return np.ascontiguousarray(out, dtype=np.float32)

